# revision 19
# baseline (speedup 1.0000x reference)
"""Trainium2 Bass kernel for BarycentricCoordinates (retrieval_knn).

Problem: template (5,8,2) f32, projections (2048,16,2) f32.
For each (v, r, a): find closest projected neighbor C of template point T,
then among all pairs {i,j} of the remaining 15 neighbors pick the valid
triangle (C,Pi,Pj) (barycentric coords of T all in [0,1], non-degenerate)
minimizing d_i + d_j + d_c; output barycentric weights + point indices.

Device algorithm (cross-product formulation, validated vs the f64
reference on the fixed seed-0 dataset; 1/81920 rows differs from a
1.3e-6-relative score near-tie, rel err 4.5e-4 << 2e-2):
  per row: d2_j = |T-P_j|^2, C = argmin, e_j = P_j - C, v2 = T - C,
  w_j = cross(e_j, v2);  pair slots (k=1..8, i=0..15), j = (i+k) mod 16:
  c = cross(e_i, e_j), al = w_j*c, be = w_i*c,
  valid <=> min(min(-be, al), c^2-(al-be), c^2-TINY) >= 0
  nscore = max(v3 * -BIG, d_i+d_j)  (valid -> d_i+d_j, invalid -> >=1e4)
  packed = (bits(nscore) & -128) | q   (monotone for positive f32)
  rmin = min over the 128 slots  ->  only output per (v,r,a).
  BIG=1e34 keeps invalid markers finite (TINY*BIG=1e4 >> max valid ~40,
  |v3|max*BIG ~ 2e36 < f32 max); the reduce runs on a f32 bitcast view
  because the DVE reduce computes in f32 internally and would round away
  the low 7 (q) bits of an i32 input.
Host decodes q -> (i,j), recomputes the barycentric weights in f64
(exactly the reference formulas) and zeroes invalid rows.
Sharding: data-parallel over V (256 rows/core, 8 cores).
Engine split per 2560-wide body op:
  V: m1, c, v1, v2, v3, nscore, packed, reduce;  G: m2, al, be, D, Y, totp;
  S: squares + sqrt.
"""
import numpy as np

V, N, R, A = 2048, 16, 5, 8
NCORES = 8
VS = V // NCORES          # 256 rows per core
NRA = R * A               # 40 (r,a) combos
G = 20                    # (r,a) groups per pass
GV = 5                    # groups whose full chain runs on the Vector slice
NH = NRA // G             # passes per vblock
NP = 128                  # pair slots: k=1..8 x i=0..15
W32 = 32                  # duplicated point width
FDPT = G * W32            # 640
FDPR = G * NP             # 2560
OUTC = NH * G             # 40 packed-argmin words per row
BIG = 1e34
TINY = 1e-30

_cache = {}


NCST = NP + 3             # qrow | and-mask | bits(TINY) | bits(-BIG)


def _consts_np():
    row = np.concatenate([
        np.arange(NP, dtype=np.int32),
        np.array([-128], dtype=np.int32),
        np.array([TINY, -BIG], dtype=np.float32).view(np.int32),
    ])
    return np.ascontiguousarray(np.broadcast_to(row, (128, NCST)))


def _legalize_waits(nc):
    """This walrus build allows only ONE embedded sync-wait per TPB
    instruction; split extra waits onto preceding same-engine no-ops."""
    import concourse.mybir as mybir
    nsplit = 0
    for fn in nc.m.functions:
        for blk in fn.blocks:
            newlist = []
            for inst in blk.instructions:
                si = inst.sync_info
                if si is not None and len(si.on_wait) > 1:
                    waits = list(si.on_wait)
                    for i, w in enumerate(waits[:-1]):
                        nop = mybir.InstNoOp(
                            name=f"{inst.name}-wsplit{i}", ins=[], outs=[])
                        nop.engine = inst.engine
                        nop.sync_info = mybir.SyncInfo(on_wait=[w], on_update=[])
                        newlist.append(nop)
                        nsplit += 1
                    inst.sync_info = mybir.SyncInfo(
                        on_wait=[waits[-1]], on_update=list(si.on_update))
                newlist.append(inst)
            blk.instructions = newlist
    return nsplit


def _build():
    if "nc" in _cache:
        return _cache["nc"]
    import concourse.bass as bass
    import concourse.mybir as mybir
    import concourse.tile as tile

    op = mybir.AluOpType
    f32 = mybir.dt.float32
    i32 = mybir.dt.int32
    AF = mybir.ActivationFunctionType
    AX = mybir.AxisListType

    nc = bass.Bass("TRN2", target_bir_lowering=False, debug=False)
    proj_d = nc.dram_tensor("proj", [VS, N, 2], f32, kind="ExternalInput")
    tpl_d = nc.dram_tensor("tpl", [128, NRA * 2], f32, kind="ExternalInput")
    cst_d = nc.dram_tensor("cst", [128, NCST], i32, kind="ExternalInput")
    out_d = nc.dram_tensor("out", [VS, OUTC], f32, kind="ExternalOutput")

    def win(t, off, dims):
        b = t if isinstance(t, bass.AP) else t[:]
        pat = [list(b.ap[0])] + [[int(s), int(n)] for s, n in dims]
        return bass.AP(b.tensor, b.offset + off, pat)

    with tile.TileContext(nc) as tc:
        with (
            tc.tile_pool(name="cpool", bufs=1) as cp,
            tc.tile_pool(name="io", bufs=2) as iop,
            tc.tile_pool(name="pt", bufs=2) as ptp,
            tc.tile_pool(name="pair", bufs=1) as pp,
            tc.tile_pool(name="sm", bufs=2) as smp,
        ):
            cb = cp.tile([128, NCST], i32, tag="cb")
            nc.sync.dma_start(cb[:], cst_d[:])
            tplB = cp.tile([128, NRA * 2], f32, tag="tplB")
            nc.sync.dma_start(tplB[:], tpl_d[:])

            pr = proj_d[:]
            pxys = {}
            outsbs = {}

            def emit_load(vb):
                pxy = iop.tile([128, 64], f32, tag="pxy", name=f"pxy{vb}")
                sl = slice(vb * 128, (vb + 1) * 128)
                nc.sync.dma_start(pxy[:, 0:16], pr[sl, :, 0])
                nc.gpsimd.tensor_copy(pxy[:, 16:32], pxy[:, 0:16])
                nc.sync.dma_start(pxy[:, 32:48], pr[sl, :, 1])
                nc.gpsimd.tensor_copy(pxy[:, 48:64], pxy[:, 32:48])
                pxys[vb] = pxy
                outsbs[vb] = iop.tile([128, OUTC], f32, tag="outsb",
                                      name=f"outsb{vb}")

            def emit_head(vb, h):
                pxy = pxys[vb]
                txs = lambda wd: win(tplB, 2 * G * h, [[2, G], [0, wd]])
                tys = lambda wd: win(tplB, 2 * G * h + 1, [[2, G], [0, wd]])
                pxw = lambda wd: win(pxy, 0, [[0, G], [1, wd]])
                pyw = lambda wd: win(pxy, 32, [[0, G], [1, wd]])

                # ---- per-point stage ([128, G, 32]) ----
                dxw = ptp.tile([128, FDPT], f32, tag="dxw")
                dyw = ptp.tile([128, FDPT], f32, tag="dyw")
                nc.gpsimd.tensor_tensor(
                    win(dxw, 0, [[W32, G], [1, W32]]), pxw(W32), txs(W32),
                    op.subtract)
                nc.gpsimd.tensor_tensor(
                    win(dyw, 0, [[W32, G], [1, W32]]), pyw(W32), tys(W32),
                    op.subtract)
                dx2 = ptp.tile([128, FDPT], f32, tag="dx2")
                dy2 = ptp.tile([128, FDPT], f32, tag="dy2")
                nc.scalar.activation(dx2[:], dxw[:], AF.Square)
                nc.scalar.activation(dy2[:], dyw[:], AF.Square)
                d2w = ptp.tile([128, FDPT], f32, tag="d2w")
                nc.vector.tensor_add(d2w[:], dx2[:], dy2[:])
                dw = ptp.tile([128, FDPT], f32, tag="dw")
                nc.scalar.activation(dw[:], d2w[:], AF.Sqrt)

                d2m = smp.tile([128, G], f32, tag="d2m")
                nc.vector.tensor_reduce(
                    d2m[:], win(d2w, 0, [[W32, G], [1, 16]]),
                    axis=AX.X, op=op.min)
                cmw = ptp.tile([128, G * 16], f32, tag="cmw")
                nc.vector.tensor_tensor(
                    win(cmw, 0, [[16, G], [1, 16]]),
                    win(d2w, 0, [[W32, G], [1, 16]]),
                    win(d2m, 0, [[1, G], [0, 16]]), op.is_equal)

                # stacked closest-point gather: [xc | yc]
                gt0 = ptp.tile([128, 2 * G * 16], f32, tag="gt0")
                nc.vector.tensor_tensor(
                    win(gt0, 0, [[G * 16, 2], [16, G], [1, 16]]),
                    win(cmw, 0, [[0, 2], [16, G], [1, 16]]),
                    win(pxy, 0, [[32, 2], [0, G], [1, 16]]), op.mult)
                xyc = smp.tile([128, 2 * G], f32, tag="xyc")
                nc.vector.tensor_reduce(
                    xyc[:], win(gt0, 0, [[G * 16, 2], [16, G], [1, 16]]),
                    axis=AX.X, op=op.add)
                xcv = xyc[:, 0:G]
                ycv = xyc[:, G:2 * G]

                # pts layout: [ex | ey | wt], each [G, 32]
                pts = ptp.tile([128, 3 * FDPT], f32, tag="pts")
                nc.vector.tensor_tensor(
                    win(pts, 0, [[W32, G], [1, W32]]), pxw(W32),
                    win(xyc, 0, [[1, G], [0, W32]]), op.subtract)
                nc.vector.tensor_tensor(
                    win(pts, FDPT, [[W32, G], [1, W32]]), pyw(W32),
                    win(xyc, G, [[1, G], [0, W32]]), op.subtract)
                v2x = smp.tile([128, G], f32, tag="v2x")
                v2y = smp.tile([128, G], f32, tag="v2y")
                nc.vector.tensor_tensor(
                    v2x[:], win(tplB, 2 * G * h, [[2, G]]), xcv, op.subtract)
                nc.vector.tensor_tensor(
                    v2y[:], win(tplB, 2 * G * h + 1, [[2, G]]), ycv, op.subtract)
                mw1 = ptp.tile([128, FDPT], f32, tag="dx2")
                mw2 = ptp.tile([128, FDPT], f32, tag="dy2")
                nc.gpsimd.tensor_tensor(
                    win(mw1, 0, [[W32, G], [1, W32]]),
                    win(pts, FDPT, [[W32, G], [1, W32]]),
                    win(v2x, 0, [[1, G], [0, W32]]), op.mult)
                nc.gpsimd.tensor_tensor(
                    win(mw2, 0, [[W32, G], [1, W32]]),
                    win(pts, 0, [[W32, G], [1, W32]]),
                    win(v2y, 0, [[1, G], [0, W32]]), op.mult)
                nc.vector.tensor_sub(pts[:, 2 * FDPT:3 * FDPT], mw1[:], mw2[:])
                return dict(pts=pts, dw=dw)

            def emit_body(vb, h, st):
                pts, dw = st["pts"], st["dw"]
                outsb = outsbs[vb]
                # ---- pair stage, group-sliced across V and G engines ----
                # each engine runs the full dependency chain on its own
                # group slice: no cross-engine hops, no stalls.
                # Pool (gpsimd) supports only add/sub/mult TT ops: it gets a
                # large slice of the pure-arithmetic ops; Vector runs its own
                # small full-chain slice plus the min/pack stages of both.
                EX, EY, WT = 0, FDPT, 2 * FDPT
                ps2s = []
                for eng, sfx, g0, ng in ((nc.vector, "V", 0, GV),
                                         (nc.gpsimd, "G", GV, G - GV)):
                    fd = ng * NP
                    ei = lambda o: win(pts, o + g0 * W32,
                                       [[W32, ng], [0, 8], [1, 16]])
                    ej = lambda o: win(pts, o + g0 * W32 + 1,
                                       [[W32, ng], [1, 8], [1, 16]])
                    pw = lambda t: win(t, 0, [[NP, ng], [16, 8], [1, 16]])

                    m1 = pp.tile([128, fd], f32, tag="m1" + sfx)
                    eng.tensor_mul(pw(m1), ei(EX), ej(EY))
                    m2 = pp.tile([128, fd], f32, tag="m2" + sfx)
                    eng.tensor_mul(pw(m2), ei(EY), ej(EX))
                    c = pp.tile([128, fd], f32, tag="c" + sfx)
                    eng.tensor_sub(c[:], m1[:], m2[:])
                    c2 = pp.tile([128, fd], f32, tag="c2" + sfx)
                    nc.scalar.activation(c2[:], c[:], AF.Square)
                    al = pp.tile([128, fd], f32, tag="al" + sfx)
                    eng.tensor_mul(pw(al), ej(WT), pw(c))
                    be = pp.tile([128, fd], f32, tag="be" + sfx)
                    eng.tensor_mul(pw(be), ei(WT), pw(c))
                    D = pp.tile([128, fd], f32, tag="D" + sfx)
                    eng.tensor_sub(D[:], al[:], be[:])
                    Y = pp.tile([128, fd], f32, tag="Y" + sfx)
                    eng.tensor_sub(Y[:], c2[:], D[:])
                    totp = pp.tile([128, fd], f32, tag="tp" + sfx)
                    eng.tensor_add(
                        pw(totp),
                        win(dw, g0 * W32, [[W32, ng], [0, 8], [1, 16]]),
                        win(dw, g0 * W32 + 1, [[W32, ng], [1, 8], [1, 16]]))
                    # min/pack chain: DVE only
                    v1 = pp.tile([128, fd], f32, tag="v1" + sfx)
                    nc.vector.scalar_tensor_tensor(
                        v1[:], be[:], -1.0, al[:], op.mult, op.min)
                    v2 = pp.tile([128, fd], f32, tag="v2" + sfx)
                    nc.vector.tensor_tensor(v2[:], v1[:], Y[:], op.min)
                    v3 = pp.tile([128, fd], f32, tag="v1" + sfx)
                    nc.vector.scalar_tensor_tensor(
                        v3[:], c2[:], TINY, v2[:], op.subtract, op.min)
                    ns = pp.tile([128, fd], f32, tag="v2" + sfx)
                    nc.vector.scalar_tensor_tensor(
                        ns[:], v3[:], -BIG, totp[:], op.mult, op.max)
                    nsi = ns[:].bitcast(i32)
                    ps2 = pp.tile([128, fd], i32, tag="ps" + sfx)
                    nc.vector.scalar_tensor_tensor(
                        win(ps2, 0, [[NP, ng], [1, NP]]),
                        win(nsi, 0, [[NP, ng], [1, NP]]),
                        cb[:, NP:NP + 1],
                        win(cb, 0, [[0, ng], [1, NP]]),
                        op.bitwise_and, op.bitwise_or)
                    ps2s.append((ps2, g0, ng))
                for ps2, g0, ng in ps2s:
                    nc.vector.tensor_reduce(
                        outsb[:, h * G + g0:h * G + g0 + ng],
                        win(ps2[:].bitcast(f32), 0, [[NP, ng], [1, NP]]),
                        axis=AX.X, op=op.min)

            def emit_store(vb):
                sl = slice(vb * 128, (vb + 1) * 128)
                nc.sync.dma_start(out_d[sl, :], outsbs[vb][:])

            # software-pipelined emission: heads run one pass ahead of bodies
            emit_load(0)
            st = {}
            st[(0, 0)] = emit_head(0, 0)
            st[(0, 1)] = emit_head(0, 1)
            emit_body(0, 0, st.pop((0, 0)))
            emit_load(1)
            st[(1, 0)] = emit_head(1, 0)
            emit_body(0, 1, st.pop((0, 1)))
            st[(1, 1)] = emit_head(1, 1)
            emit_store(0)
            emit_body(1, 0, st.pop((1, 0)))
            emit_body(1, 1, st.pop((1, 1)))
            emit_store(1)

    _cache["nc"] = nc
    return nc


def _in_maps(template, projections):
    tpl = np.ascontiguousarray(np.broadcast_to(
        np.asarray(template, dtype=np.float32).reshape(NRA * 2), (128, NRA * 2)))
    cst = _consts_np()
    maps = []
    for k in range(NCORES):
        shard = np.ascontiguousarray(
            projections[k * VS:(k + 1) * VS], dtype=np.float32)
        maps.append({"proj": shard, "tpl": tpl, "cst": cst})
    return maps


def _decode(raw, template, projections):
    """raw: [V, 40] f32(bits) packed-argmin records -> (weights, indices)."""
    rmin = np.ascontiguousarray(
        np.asarray(raw, dtype=np.float32)).view(np.int32).reshape(V, R, A)
    q = rmin & 127
    nsc = (rmin & ~np.int32(127)).view(np.float32)
    flag = nsc < 1e3

    k_sel = (q >> 4) + 1
    i_sel = q & 15
    j_sel = (i_sel + k_sel) % 16

    proj64 = np.asarray(projections, dtype=np.float64)
    tpl64 = np.asarray(template, dtype=np.float64)
    px64 = proj64[:, :, 0]
    py64 = proj64[:, :, 1]

    dxa = tpl64[None, :, :, 0:1] - px64[:, None, None, :]
    dya = tpl64[None, :, :, 1:2] - py64[:, None, None, :]
    dall = np.sqrt(dxa * dxa + dya * dya)            # (V,R,A,16)
    cidx = np.argmin(dall, axis=-1)                  # reference order[...,0]

    d_i = np.take_along_axis(dall, i_sel[..., None], -1)[..., 0]
    d_j = np.take_along_axis(dall, j_sel[..., None], -1)[..., 0]
    swap = (d_j < d_i) | ((d_j == d_i) & (j_sel < i_sel))
    first = np.where(swap, j_sel, i_sel)
    second = np.where(swap, i_sel, j_sel)

    def gat(arr, idx):
        return np.take_along_axis(arr, idx.reshape(V, -1), -1).reshape(V, R, A)

    Cx, Cy = gat(px64, cidx), gat(py64, cidx)
    e1x = gat(px64, first) - Cx
    e1y = gat(py64, first) - Cy
    e2x = gat(px64, second) - Cx
    e2y = gat(py64, second) - Cy
    vtx = tpl64[None, :, :, 0] - Cx
    vty = tpl64[None, :, :, 1] - Cy
    dot00 = e1x * e1x + e1y * e1y
    dot11 = e2x * e2x + e2y * e2y
    dot01 = e1x * e2x + e1y * e2y
    dot02 = e1x * vtx + e1y * vty
    dot12 = e2x * vtx + e2y * vty
    denom = dot00 * dot11 - dot01 * dot01
    with np.errstate(divide="ignore", invalid="ignore"):
        dinv = np.where(denom == 0.0, 0.0,
                        1.0 / np.where(denom == 0.0, 1.0, denom))
    p2 = (dot02 * dot11 - dot01 * dot12) * dinv
    p1 = (dot00 * dot12 - dot01 * dot02) * dinv
    p0 = 1.0 - p2 - p1

    ok = (flag & np.isfinite(p0) & np.isfinite(p1) & np.isfinite(p2)
          & (denom != 0.0))
    weights = np.zeros((V, R, A, 3), np.float32)
    indices = np.zeros((V, R, A, 3), np.int32)
    weights[..., 0] = np.where(ok, p0, 0).astype(np.float32)
    weights[..., 1] = np.where(ok, p2, 0).astype(np.float32)
    weights[..., 2] = np.where(ok, p1, 0).astype(np.float32)
    indices[..., 0] = np.where(ok, cidx, 0)
    indices[..., 1] = np.where(ok, first, 0)
    indices[..., 2] = np.where(ok, second, 0)
    return weights, indices


def _run_device(template, projections, trace=False, **kwargs):
    from concourse.bass_utils import run_bass_kernel_spmd
    nc = _build()
    if not _cache.get("legalized"):
        _legalize_waits(nc)
        _cache["legalized"] = True
    maps = _in_maps(template, projections)
    res = run_bass_kernel_spmd(nc, maps, core_ids=list(range(NCORES)),
                               trace=trace, **kwargs)
    raw = np.concatenate([r["out"] for r in res.results], axis=0)  # [V, 40]
    return raw, res


def kernel(template, projections):
    template = np.asarray(template, dtype=np.float32)
    projections = np.asarray(projections, dtype=np.float32)
    raw, _ = _run_device(template, projections, trace=False)
    return _decode(raw, template, projections)


# revision 22
# speedup vs baseline: 1.3256x; 1.3256x over previous
"""Trainium2 Bass kernel for BarycentricCoordinates (retrieval_knn).

Problem: template (5,8,2) f32, projections (2048,16,2) f32.
For each (v, r, a): find closest projected neighbor C of template point T,
then among all pairs {i,j} of the remaining 15 neighbors pick the valid
triangle (C,Pi,Pj) (barycentric coords of T all in [0,1], non-degenerate)
minimizing d_i + d_j + d_c; output barycentric weights + point indices.

Device algorithm (cross-product formulation, validated vs the f64
reference on the fixed seed-0 dataset; 1/81920 rows differs from a
1.3e-6-relative score near-tie, rel err 4.5e-4 << 2e-2):
  per row: d2_j = |T-P_j|^2, C = argmin, e_j = P_j - C, v2 = T - C,
  w_j = cross(e_j, v2);  pair slots (k=1..8, i=0..15), j = (i+k) mod 16:
  c = cross(e_i, e_j), al = w_j*c, be = w_i*c,
  valid <=> min(min(-be, al), c^2-(al-be), c^2-TINY) >= 0
  nscore = max(v3 * -BIG, d_i+d_j)  (valid -> d_i+d_j, invalid -> >=1e4)
  packed = (bits(nscore) & -128) | q   (monotone for positive f32)
  rmin = min over the 128 slots  ->  only output per (v,r,a).
  BIG=1e34 keeps invalid markers finite (TINY*BIG=1e4 >> max valid ~40,
  |v3|max*BIG ~ 2e36 < f32 max); the reduce runs on a f32 bitcast view
  because the DVE reduce computes in f32 internally and would round away
  the low 7 (q) bits of an i32 input.
Host decodes q -> (i,j), recomputes the barycentric weights in f64
(exactly the reference formulas) and zeroes invalid rows.
Sharding: data-parallel over V (256 rows/core, 8 cores).
Engine split per 2560-wide body op:
  V: m1, c, v1, v2, v3, nscore, packed, reduce;  G: m2, al, be, D, Y, totp;
  S: squares + sqrt.
"""
import numpy as np

V, N, R, A = 2048, 16, 5, 8
NCORES = 8
VS = V // NCORES          # 256 rows per core
NRA = R * A               # 40 (r,a) combos
G = 20                    # (r,a) groups per pass
GV = 5                    # groups whose full chain runs on the Vector slice
NH = NRA // G             # passes per vblock
NP = 128                  # pair slots: k=1..8 x i=0..15
W32 = 32                  # duplicated point width
FDPT = G * W32            # 640
FDPR = G * NP             # 2560
OUTC = NH * G             # 40 packed-argmin words per row
BIG = 1e34
TINY = 1e-30

_cache = {}


NCST = NP + 3             # qrow | and-mask | bits(TINY) | bits(-BIG)


def _consts_np():
    row = np.concatenate([
        np.arange(NP, dtype=np.int32),
        np.array([-128], dtype=np.int32),
        np.array([TINY, -BIG], dtype=np.float32).view(np.int32),
    ])
    return np.ascontiguousarray(np.broadcast_to(row, (128, NCST)))


def _legalize_waits(nc):
    """This walrus build allows only ONE embedded sync-wait per TPB
    instruction; split extra waits onto preceding same-engine no-ops."""
    import concourse.mybir as mybir
    nsplit = 0
    for fn in nc.m.functions:
        for blk in fn.blocks:
            newlist = []
            for inst in blk.instructions:
                si = inst.sync_info
                if si is not None and len(si.on_wait) > 1:
                    waits = list(si.on_wait)
                    for i, w in enumerate(waits[:-1]):
                        nop = mybir.InstNoOp(
                            name=f"{inst.name}-wsplit{i}", ins=[], outs=[])
                        nop.engine = inst.engine
                        nop.sync_info = mybir.SyncInfo(on_wait=[w], on_update=[])
                        newlist.append(nop)
                        nsplit += 1
                    inst.sync_info = mybir.SyncInfo(
                        on_wait=[waits[-1]], on_update=list(si.on_update))
                newlist.append(inst)
            blk.instructions = newlist
    return nsplit


def _build():
    if "nc" in _cache:
        return _cache["nc"]
    import concourse.bass as bass
    import concourse.mybir as mybir
    import concourse.tile as tile

    op = mybir.AluOpType
    f32 = mybir.dt.float32
    i32 = mybir.dt.int32
    AF = mybir.ActivationFunctionType
    AX = mybir.AxisListType

    nc = bass.Bass("TRN2", target_bir_lowering=False, debug=False)
    proj_d = nc.dram_tensor("proj", [VS, N, 2], f32, kind="ExternalInput")
    tpl_d = nc.dram_tensor("tpl", [128, NRA * 2], f32, kind="ExternalInput")
    cst_d = nc.dram_tensor("cst", [128, NCST], i32, kind="ExternalInput")
    out_d = nc.dram_tensor("out", [VS, OUTC], f32, kind="ExternalOutput")

    def win(t, off, dims):
        b = t if isinstance(t, bass.AP) else t[:]
        pat = [list(b.ap[0])] + [[int(s), int(n)] for s, n in dims]
        return bass.AP(b.tensor, b.offset + off, pat)

    with tile.TileContext(nc) as tc:
        with (
            tc.tile_pool(name="cpool", bufs=1) as cp,
            tc.tile_pool(name="io", bufs=2) as iop,
            tc.tile_pool(name="pt", bufs=2) as ptp,
            tc.tile_pool(name="pair", bufs=1) as pp,
            tc.tile_pool(name="sm", bufs=2) as smp,
        ):
            cb = cp.tile([128, NCST], i32, tag="cb")
            nc.sync.dma_start(cb[:], cst_d[:])
            tplB = cp.tile([128, NRA * 2], f32, tag="tplB")
            nc.sync.dma_start(tplB[:], tpl_d[:])

            pr = proj_d[:]
            pxys = {}
            outsbs = {}

            def emit_load(vb):
                pxy = iop.tile([128, 64], f32, tag="pxy", name=f"pxy{vb}")
                sl = slice(vb * 128, (vb + 1) * 128)
                nc.sync.dma_start(pxy[:, 0:16], pr[sl, :, 0])
                nc.gpsimd.tensor_copy(pxy[:, 16:32], pxy[:, 0:16])
                nc.sync.dma_start(pxy[:, 32:48], pr[sl, :, 1])
                nc.gpsimd.tensor_copy(pxy[:, 48:64], pxy[:, 32:48])
                pxys[vb] = pxy
                outsbs[vb] = iop.tile([128, OUTC], f32, tag="outsb",
                                      name=f"outsb{vb}")

            def emit_head(vb, h):
                pxy = pxys[vb]
                txs = lambda wd: win(tplB, 2 * G * h, [[2, G], [0, wd]])
                tys = lambda wd: win(tplB, 2 * G * h + 1, [[2, G], [0, wd]])
                pxw = lambda wd: win(pxy, 0, [[0, G], [1, wd]])
                pyw = lambda wd: win(pxy, 32, [[0, G], [1, wd]])

                # ---- per-point stage ([128, G, 32]) ----
                dxw = ptp.tile([128, FDPT], f32, tag="dxw")
                dyw = ptp.tile([128, FDPT], f32, tag="dyw")
                nc.vector.tensor_tensor(
                    win(dxw, 0, [[W32, G], [1, W32]]), pxw(W32), txs(W32),
                    op.subtract)
                nc.vector.tensor_tensor(
                    win(dyw, 0, [[W32, G], [1, W32]]), pyw(W32), tys(W32),
                    op.subtract)
                dx2 = ptp.tile([128, FDPT], f32, tag="dx2")
                dy2 = ptp.tile([128, FDPT], f32, tag="dy2")
                nc.scalar.activation(dx2[:], dxw[:], AF.Square)
                nc.scalar.activation(dy2[:], dyw[:], AF.Square)
                d2w = ptp.tile([128, FDPT], f32, tag="d2w")
                nc.vector.tensor_add(d2w[:], dx2[:], dy2[:])
                dw = ptp.tile([128, FDPT], f32, tag="dw")
                nc.scalar.activation(dw[:], d2w[:], AF.Sqrt)

                d2m = smp.tile([128, G], f32, tag="d2m")
                nc.vector.tensor_reduce(
                    d2m[:], win(d2w, 0, [[W32, G], [1, 16]]),
                    axis=AX.X, op=op.min)
                cmw = ptp.tile([128, G * 16], f32, tag="cmw")
                nc.vector.tensor_tensor(
                    win(cmw, 0, [[16, G], [1, 16]]),
                    win(d2w, 0, [[W32, G], [1, 16]]),
                    win(d2m, 0, [[1, G], [0, 16]]), op.is_equal)

                # stacked closest-point gather: [xc | yc]
                gt0 = ptp.tile([128, 2 * G * 16], f32, tag="gt0")
                nc.vector.tensor_tensor(
                    win(gt0, 0, [[G * 16, 2], [16, G], [1, 16]]),
                    win(cmw, 0, [[0, 2], [16, G], [1, 16]]),
                    win(pxy, 0, [[32, 2], [0, G], [1, 16]]), op.mult)
                xyc = smp.tile([128, 2 * G], f32, tag="xyc")
                nc.vector.tensor_reduce(
                    xyc[:], win(gt0, 0, [[G * 16, 2], [16, G], [1, 16]]),
                    axis=AX.X, op=op.add)
                xcv = xyc[:, 0:G]
                ycv = xyc[:, G:2 * G]

                # pts layout: [ex | ey | wt], each [G, 32]
                pts = ptp.tile([128, 3 * FDPT], f32, tag="pts")
                nc.vector.tensor_tensor(
                    win(pts, 0, [[W32, G], [1, W32]]), pxw(W32),
                    win(xyc, 0, [[1, G], [0, W32]]), op.subtract)
                nc.vector.tensor_tensor(
                    win(pts, FDPT, [[W32, G], [1, W32]]), pyw(W32),
                    win(xyc, G, [[1, G], [0, W32]]), op.subtract)
                v2x = smp.tile([128, G], f32, tag="v2x")
                v2y = smp.tile([128, G], f32, tag="v2y")
                nc.vector.tensor_tensor(
                    v2x[:], win(tplB, 2 * G * h, [[2, G]]), xcv, op.subtract)
                nc.vector.tensor_tensor(
                    v2y[:], win(tplB, 2 * G * h + 1, [[2, G]]), ycv, op.subtract)
                mw1 = ptp.tile([128, FDPT], f32, tag="dx2")
                mw2 = ptp.tile([128, FDPT], f32, tag="dy2")
                nc.vector.tensor_tensor(
                    win(mw1, 0, [[W32, G], [1, W32]]),
                    win(pts, FDPT, [[W32, G], [1, W32]]),
                    win(v2x, 0, [[1, G], [0, W32]]), op.mult)
                nc.vector.tensor_tensor(
                    win(mw2, 0, [[W32, G], [1, W32]]),
                    win(pts, 0, [[W32, G], [1, W32]]),
                    win(v2y, 0, [[1, G], [0, W32]]), op.mult)
                nc.vector.tensor_sub(pts[:, 2 * FDPT:3 * FDPT], mw1[:], mw2[:])
                return dict(pts=pts, dw=dw)

            def emit_body(vb, h, st):
                pts, dw = st["pts"], st["dw"]
                outsb = outsbs[vb]
                # ---- pair stage, group-sliced across V and G engines ----
                # each engine runs the full dependency chain on its own
                # group slice: no cross-engine hops, no stalls.
                # Concurrent Pool-engine activity starves the DVE's
                # short-burst windowed reads (measured 6x slowdown), so the
                # whole pair stage runs on Vector, squares on Scalar.
                EX, EY, WT = 0, FDPT, 2 * FDPT
                ps2s = []
                for eng, sfx, g0, ng in ((nc.vector, "V", 0, G),):
                    fd = ng * NP
                    ei = lambda o: win(pts, o + g0 * W32,
                                       [[W32, ng], [0, 8], [1, 16]])
                    ej = lambda o: win(pts, o + g0 * W32 + 1,
                                       [[W32, ng], [1, 8], [1, 16]])
                    pw = lambda t: win(t, 0, [[NP, ng], [16, 8], [1, 16]])

                    m1 = pp.tile([128, fd], f32, tag="m1" + sfx)
                    eng.tensor_mul(pw(m1), ei(EX), ej(EY))
                    m2 = pp.tile([128, fd], f32, tag="m2" + sfx)
                    eng.tensor_mul(pw(m2), ei(EY), ej(EX))
                    c = pp.tile([128, fd], f32, tag="c" + sfx)
                    eng.tensor_sub(c[:], m1[:], m2[:])
                    c2 = pp.tile([128, fd], f32, tag="c2" + sfx)
                    nc.scalar.activation(c2[:], c[:], AF.Square)
                    al = pp.tile([128, fd], f32, tag="al" + sfx)
                    eng.tensor_mul(pw(al), ej(WT), pw(c))
                    be = pp.tile([128, fd], f32, tag="be" + sfx)
                    eng.tensor_mul(pw(be), ei(WT), pw(c))
                    D = pp.tile([128, fd], f32, tag="D" + sfx)
                    eng.tensor_sub(D[:], al[:], be[:])
                    Y = pp.tile([128, fd], f32, tag="Y" + sfx)
                    eng.tensor_sub(Y[:], c2[:], D[:])
                    totp = pp.tile([128, fd], f32, tag="tp" + sfx)
                    eng.tensor_add(
                        pw(totp),
                        win(dw, g0 * W32, [[W32, ng], [0, 8], [1, 16]]),
                        win(dw, g0 * W32 + 1, [[W32, ng], [1, 8], [1, 16]]))
                    # min/pack chain: DVE only
                    v1 = pp.tile([128, fd], f32, tag="v1" + sfx)
                    nc.vector.scalar_tensor_tensor(
                        v1[:], be[:], -1.0, al[:], op.mult, op.min)
                    v2 = pp.tile([128, fd], f32, tag="v2" + sfx)
                    nc.vector.tensor_tensor(v2[:], v1[:], Y[:], op.min)
                    v3 = pp.tile([128, fd], f32, tag="v1" + sfx)
                    nc.vector.scalar_tensor_tensor(
                        v3[:], c2[:], TINY, v2[:], op.subtract, op.min)
                    ns = pp.tile([128, fd], f32, tag="v2" + sfx)
                    nc.vector.scalar_tensor_tensor(
                        ns[:], v3[:], -BIG, totp[:], op.mult, op.max)
                    nsi = ns[:].bitcast(i32)
                    ps2 = pp.tile([128, fd], i32, tag="ps" + sfx)
                    nc.vector.scalar_tensor_tensor(
                        win(ps2, 0, [[NP, ng], [1, NP]]),
                        win(nsi, 0, [[NP, ng], [1, NP]]),
                        cb[:, NP:NP + 1],
                        win(cb, 0, [[0, ng], [1, NP]]),
                        op.bitwise_and, op.bitwise_or)
                    ps2s.append((ps2, g0, ng))
                for ps2, g0, ng in ps2s:
                    nc.vector.tensor_reduce(
                        outsb[:, h * G + g0:h * G + g0 + ng],
                        win(ps2[:].bitcast(f32), 0, [[NP, ng], [1, NP]]),
                        axis=AX.X, op=op.min)

            def emit_store(vb):
                sl = slice(vb * 128, (vb + 1) * 128)
                nc.sync.dma_start(out_d[sl, :], outsbs[vb][:])

            # software-pipelined emission: heads run one pass ahead of bodies
            emit_load(0)
            st = {}
            st[(0, 0)] = emit_head(0, 0)
            st[(0, 1)] = emit_head(0, 1)
            emit_body(0, 0, st.pop((0, 0)))
            emit_load(1)
            st[(1, 0)] = emit_head(1, 0)
            emit_body(0, 1, st.pop((0, 1)))
            st[(1, 1)] = emit_head(1, 1)
            emit_store(0)
            emit_body(1, 0, st.pop((1, 0)))
            emit_body(1, 1, st.pop((1, 1)))
            emit_store(1)

    _cache["nc"] = nc
    return nc


def _in_maps(template, projections):
    tpl = np.ascontiguousarray(np.broadcast_to(
        np.asarray(template, dtype=np.float32).reshape(NRA * 2), (128, NRA * 2)))
    cst = _consts_np()
    maps = []
    for k in range(NCORES):
        shard = np.ascontiguousarray(
            projections[k * VS:(k + 1) * VS], dtype=np.float32)
        maps.append({"proj": shard, "tpl": tpl, "cst": cst})
    return maps


def _decode(raw, template, projections):
    """raw: [V, 40] f32(bits) packed-argmin records -> (weights, indices)."""
    rmin = np.ascontiguousarray(
        np.asarray(raw, dtype=np.float32)).view(np.int32).reshape(V, R, A)
    q = rmin & 127
    nsc = (rmin & ~np.int32(127)).view(np.float32)
    flag = nsc < 1e3

    k_sel = (q >> 4) + 1
    i_sel = q & 15
    j_sel = (i_sel + k_sel) % 16

    proj64 = np.asarray(projections, dtype=np.float64)
    tpl64 = np.asarray(template, dtype=np.float64)
    px64 = proj64[:, :, 0]
    py64 = proj64[:, :, 1]

    dxa = tpl64[None, :, :, 0:1] - px64[:, None, None, :]
    dya = tpl64[None, :, :, 1:2] - py64[:, None, None, :]
    dall = np.sqrt(dxa * dxa + dya * dya)            # (V,R,A,16)
    cidx = np.argmin(dall, axis=-1)                  # reference order[...,0]

    d_i = np.take_along_axis(dall, i_sel[..., None], -1)[..., 0]
    d_j = np.take_along_axis(dall, j_sel[..., None], -1)[..., 0]
    swap = (d_j < d_i) | ((d_j == d_i) & (j_sel < i_sel))
    first = np.where(swap, j_sel, i_sel)
    second = np.where(swap, i_sel, j_sel)

    def gat(arr, idx):
        return np.take_along_axis(arr, idx.reshape(V, -1), -1).reshape(V, R, A)

    Cx, Cy = gat(px64, cidx), gat(py64, cidx)
    e1x = gat(px64, first) - Cx
    e1y = gat(py64, first) - Cy
    e2x = gat(px64, second) - Cx
    e2y = gat(py64, second) - Cy
    vtx = tpl64[None, :, :, 0] - Cx
    vty = tpl64[None, :, :, 1] - Cy
    dot00 = e1x * e1x + e1y * e1y
    dot11 = e2x * e2x + e2y * e2y
    dot01 = e1x * e2x + e1y * e2y
    dot02 = e1x * vtx + e1y * vty
    dot12 = e2x * vtx + e2y * vty
    denom = dot00 * dot11 - dot01 * dot01
    with np.errstate(divide="ignore", invalid="ignore"):
        dinv = np.where(denom == 0.0, 0.0,
                        1.0 / np.where(denom == 0.0, 1.0, denom))
    p2 = (dot02 * dot11 - dot01 * dot12) * dinv
    p1 = (dot00 * dot12 - dot01 * dot02) * dinv
    p0 = 1.0 - p2 - p1

    ok = (flag & np.isfinite(p0) & np.isfinite(p1) & np.isfinite(p2)
          & (denom != 0.0))
    weights = np.zeros((V, R, A, 3), np.float32)
    indices = np.zeros((V, R, A, 3), np.int32)
    weights[..., 0] = np.where(ok, p0, 0).astype(np.float32)
    weights[..., 1] = np.where(ok, p2, 0).astype(np.float32)
    weights[..., 2] = np.where(ok, p1, 0).astype(np.float32)
    indices[..., 0] = np.where(ok, cidx, 0)
    indices[..., 1] = np.where(ok, first, 0)
    indices[..., 2] = np.where(ok, second, 0)
    return weights, indices


def _run_device(template, projections, trace=False, **kwargs):
    from concourse.bass_utils import run_bass_kernel_spmd
    nc = _build()
    if not _cache.get("legalized"):
        _legalize_waits(nc)
        _cache["legalized"] = True
    maps = _in_maps(template, projections)
    res = run_bass_kernel_spmd(nc, maps, core_ids=list(range(NCORES)),
                               trace=trace, **kwargs)
    raw = np.concatenate([r["out"] for r in res.results], axis=0)  # [V, 40]
    return raw, res


def kernel(template, projections):
    template = np.asarray(template, dtype=np.float32)
    projections = np.asarray(projections, dtype=np.float32)
    raw, _ = _run_device(template, projections, trace=False)
    return _decode(raw, template, projections)


# revision 24
# speedup vs baseline: 1.3598x; 1.0258x over previous
"""Trainium2 Bass kernel for BarycentricCoordinates (retrieval_knn).

Problem: template (5,8,2) f32, projections (2048,16,2) f32.
For each (v, r, a): find closest projected neighbor C of template point T,
then among all pairs {i,j} of the remaining 15 neighbors pick the valid
triangle (C,Pi,Pj) (barycentric coords of T all in [0,1], non-degenerate)
minimizing d_i + d_j + d_c; output barycentric weights + point indices.

Device algorithm (cross-product formulation, validated vs the f64
reference on the fixed seed-0 dataset; 1/81920 rows differs from a
1.3e-6-relative score near-tie, rel err 4.5e-4 << 2e-2):
  per row: d2_j = |T-P_j|^2, C = argmin, e_j = P_j - C, v2 = T - C,
  w_j = cross(e_j, v2);  pair slots (k=1..8, i=0..15), j = (i+k) mod 16:
  c = cross(e_i, e_j), al = w_j*c, be = w_i*c,
  valid <=> min(min(-be, al), c^2-(al-be), c^2-TINY) >= 0
  nscore = max(v3 * -BIG, d_i+d_j)  (valid -> d_i+d_j, invalid -> >=1e4)
  packed = (bits(nscore) & -128) | q   (monotone for positive f32)
  rmin = min over the 128 slots  ->  only output per (v,r,a).
  BIG=1e34 keeps invalid markers finite (TINY*BIG=1e4 >> max valid ~40,
  |v3|max*BIG ~ 2e36 < f32 max); the reduce runs on a f32 bitcast view
  because the DVE reduce computes in f32 internally and would round away
  the low 7 (q) bits of an i32 input.
Host decodes q -> (i,j), recomputes the barycentric weights in f64
(exactly the reference formulas) and zeroes invalid rows.
Sharding: data-parallel over V (256 rows/core, 8 cores).
Engine split per 2560-wide body op:
  V: m1, c, v1, v2, v3, nscore, packed, reduce;  G: m2, al, be, D, Y, totp;
  S: squares + sqrt.
"""
import numpy as np

V, N, R, A = 2048, 16, 5, 8
NCORES = 8
VS = V // NCORES          # 256 rows per core
NRA = R * A               # 40 (r,a) combos
G = 20                    # (r,a) groups per pass
GV = 5                    # groups whose full chain runs on the Vector slice
NH = NRA // G             # passes per vblock
NP = 128                  # pair slots: k=1..8 x i=0..15
W32 = 32                  # duplicated point width
FDPT = G * W32            # 640
FDPR = G * NP             # 2560
OUTC = NH * G             # 40 packed-argmin words per row
BIG = 1e34
TINY = 1e-30

_cache = {}


NCST = NP + 3             # qrow | and-mask | bits(TINY) | bits(-BIG)


def _consts_np():
    row = np.concatenate([
        np.arange(NP, dtype=np.int32),
        np.array([-128], dtype=np.int32),
        np.array([TINY, -BIG], dtype=np.float32).view(np.int32),
    ])
    return np.ascontiguousarray(np.broadcast_to(row, (128, NCST)))


def _legalize_waits(nc):
    """This walrus build allows only ONE embedded sync-wait per TPB
    instruction; split extra waits onto preceding same-engine no-ops."""
    import concourse.mybir as mybir
    nsplit = 0
    for fn in nc.m.functions:
        for blk in fn.blocks:
            newlist = []
            for inst in blk.instructions:
                si = inst.sync_info
                if si is not None and len(si.on_wait) > 1:
                    waits = list(si.on_wait)
                    for i, w in enumerate(waits[:-1]):
                        nop = mybir.InstNoOp(
                            name=f"{inst.name}-wsplit{i}", ins=[], outs=[])
                        nop.engine = inst.engine
                        nop.sync_info = mybir.SyncInfo(on_wait=[w], on_update=[])
                        newlist.append(nop)
                        nsplit += 1
                    inst.sync_info = mybir.SyncInfo(
                        on_wait=[waits[-1]], on_update=list(si.on_update))
                newlist.append(inst)
            blk.instructions = newlist
    return nsplit


def _build():
    if "nc" in _cache:
        return _cache["nc"]
    import concourse.bass as bass
    import concourse.mybir as mybir
    import concourse.tile as tile

    op = mybir.AluOpType
    f32 = mybir.dt.float32
    i32 = mybir.dt.int32
    AF = mybir.ActivationFunctionType
    AX = mybir.AxisListType

    nc = bass.Bass("TRN2", target_bir_lowering=False, debug=False)
    proj_d = nc.dram_tensor("proj", [VS, N, 2], f32, kind="ExternalInput")
    tpl_d = nc.dram_tensor("tpl", [128, NRA * 2], f32, kind="ExternalInput")
    cst_d = nc.dram_tensor("cst", [128, NCST], i32, kind="ExternalInput")
    out_d = nc.dram_tensor("out", [VS, OUTC], f32, kind="ExternalOutput")

    def win(t, off, dims):
        b = t if isinstance(t, bass.AP) else t[:]
        pat = [list(b.ap[0])] + [[int(s), int(n)] for s, n in dims]
        return bass.AP(b.tensor, b.offset + off, pat)

    with tile.TileContext(nc) as tc:
        with (
            tc.tile_pool(name="cpool", bufs=1) as cp,
            tc.tile_pool(name="io", bufs=2) as iop,
            tc.tile_pool(name="pt", bufs=2) as ptp,
            tc.tile_pool(name="pair", bufs=1) as pp,
            tc.tile_pool(name="sm", bufs=2) as smp,
        ):
            cb = cp.tile([128, NCST], i32, tag="cb")
            nc.sync.dma_start(cb[:], cst_d[:])
            tplB = cp.tile([128, NRA * 2], f32, tag="tplB")
            nc.sync.dma_start(tplB[:], tpl_d[:])

            pr = proj_d[:]
            pxys = {}
            outsbs = {}

            def emit_load(vb):
                pxy = iop.tile([128, 64], f32, tag="pxy", name=f"pxy{vb}")
                sl = slice(vb * 128, (vb + 1) * 128)
                nc.sync.dma_start(pxy[:, 0:16], pr[sl, :, 0])
                nc.gpsimd.tensor_copy(pxy[:, 16:32], pxy[:, 0:16])
                nc.sync.dma_start(pxy[:, 32:48], pr[sl, :, 1])
                nc.gpsimd.tensor_copy(pxy[:, 48:64], pxy[:, 32:48])
                pxys[vb] = pxy
                outsbs[vb] = iop.tile([128, OUTC], f32, tag="outsb",
                                      name=f"outsb{vb}")

            def emit_head(vb, h):
                pxy = pxys[vb]
                txs = lambda wd: win(tplB, 2 * G * h, [[2, G], [0, wd]])
                tys = lambda wd: win(tplB, 2 * G * h + 1, [[2, G], [0, wd]])
                pxw = lambda wd: win(pxy, 0, [[0, G], [1, wd]])
                pyw = lambda wd: win(pxy, 32, [[0, G], [1, wd]])

                # ---- per-point stage: distance chain at 16/group (320) ----
                F16 = G * 16
                dxw = ptp.tile([128, F16], f32, tag="dxw")
                dyw = ptp.tile([128, F16], f32, tag="dyw")
                nc.vector.tensor_tensor(
                    win(dxw, 0, [[16, G], [1, 16]]), pxw(16), txs(16),
                    op.subtract)
                nc.vector.tensor_tensor(
                    win(dyw, 0, [[16, G], [1, 16]]), pyw(16), tys(16),
                    op.subtract)
                dx2 = ptp.tile([128, F16], f32, tag="dx2")
                dy2 = ptp.tile([128, F16], f32, tag="dy2")
                nc.scalar.activation(dx2[:], dxw[:], AF.Square)
                nc.scalar.activation(dy2[:], dyw[:], AF.Square)
                d2w = ptp.tile([128, F16], f32, tag="d2w")
                nc.vector.tensor_add(d2w[:], dx2[:], dy2[:])
                dw = ptp.tile([128, FDPT], f32, tag="dw")
                nc.scalar.activation(
                    win(dw, 0, [[W32, G], [1, 16]]),
                    win(d2w, 0, [[16, G], [1, 16]]), AF.Sqrt)
                nc.scalar.activation(
                    win(dw, 16, [[W32, G], [1, 16]]),
                    win(d2w, 0, [[16, G], [1, 16]]), AF.Sqrt)

                d2m = smp.tile([128, G], f32, tag="d2m")
                nc.vector.tensor_reduce(
                    d2m[:], win(d2w, 0, [[16, G], [1, 16]]),
                    axis=AX.X, op=op.min)
                cmw = ptp.tile([128, G * 16], f32, tag="cmw")
                nc.vector.tensor_tensor(
                    win(cmw, 0, [[16, G], [1, 16]]),
                    win(d2w, 0, [[16, G], [1, 16]]),
                    win(d2m, 0, [[1, G], [0, 16]]), op.is_equal)

                # stacked closest-point gather: [xc | yc]
                gt0 = ptp.tile([128, 2 * G * 16], f32, tag="gt0")
                nc.vector.tensor_tensor(
                    win(gt0, 0, [[G * 16, 2], [16, G], [1, 16]]),
                    win(cmw, 0, [[0, 2], [16, G], [1, 16]]),
                    win(pxy, 0, [[32, 2], [0, G], [1, 16]]), op.mult)
                xyc = smp.tile([128, 2 * G], f32, tag="xyc")
                nc.vector.tensor_reduce(
                    xyc[:], win(gt0, 0, [[G * 16, 2], [16, G], [1, 16]]),
                    axis=AX.X, op=op.add)
                xcv = xyc[:, 0:G]
                ycv = xyc[:, G:2 * G]

                # pts layout: [ex | ey | wt], each [G, 32]
                pts = ptp.tile([128, 3 * FDPT], f32, tag="pts")
                nc.vector.tensor_tensor(
                    win(pts, 0, [[W32, G], [1, W32]]), pxw(W32),
                    win(xyc, 0, [[1, G], [0, W32]]), op.subtract)
                nc.vector.tensor_tensor(
                    win(pts, FDPT, [[W32, G], [1, W32]]), pyw(W32),
                    win(xyc, G, [[1, G], [0, W32]]), op.subtract)
                v2x = smp.tile([128, G], f32, tag="v2x")
                v2y = smp.tile([128, G], f32, tag="v2y")
                nc.vector.tensor_tensor(
                    v2x[:], win(tplB, 2 * G * h, [[2, G]]), xcv, op.subtract)
                nc.vector.tensor_tensor(
                    v2y[:], win(tplB, 2 * G * h + 1, [[2, G]]), ycv, op.subtract)
                mw1 = ptp.tile([128, F16], f32, tag="dx2")
                mw2 = ptp.tile([128, F16], f32, tag="dy2")
                nc.vector.tensor_tensor(
                    win(mw1, 0, [[16, G], [1, 16]]),
                    win(pts, FDPT, [[W32, G], [1, 16]]),
                    win(v2x, 0, [[1, G], [0, 16]]), op.mult)
                nc.vector.tensor_tensor(
                    win(mw2, 0, [[16, G], [1, 16]]),
                    win(pts, 0, [[W32, G], [1, 16]]),
                    win(v2y, 0, [[1, G], [0, 16]]), op.mult)
                nc.vector.tensor_tensor(
                    win(pts, 2 * FDPT, [[W32, G], [1, 16]]),
                    win(mw1, 0, [[16, G], [1, 16]]),
                    win(mw2, 0, [[16, G], [1, 16]]), op.subtract)
                nc.vector.tensor_tensor(
                    win(pts, 2 * FDPT + 16, [[W32, G], [1, 16]]),
                    win(mw1, 0, [[16, G], [1, 16]]),
                    win(mw2, 0, [[16, G], [1, 16]]), op.subtract)
                return dict(pts=pts, dw=dw)

            def emit_body(vb, h, st):
                pts, dw = st["pts"], st["dw"]
                outsb = outsbs[vb]
                # ---- pair stage, group-sliced across V and G engines ----
                # each engine runs the full dependency chain on its own
                # group slice: no cross-engine hops, no stalls.
                # Concurrent Pool-engine activity starves the DVE's
                # short-burst windowed reads (measured 6x slowdown), so the
                # whole pair stage runs on Vector, squares on Scalar.
                EX, EY, WT = 0, FDPT, 2 * FDPT
                ps2s = []
                for eng, sfx, g0, ng in ((nc.vector, "V", 0, G),):
                    fd = ng * NP
                    ei = lambda o: win(pts, o + g0 * W32,
                                       [[W32, ng], [0, 8], [1, 16]])
                    ej = lambda o: win(pts, o + g0 * W32 + 1,
                                       [[W32, ng], [1, 8], [1, 16]])
                    pw = lambda t: win(t, 0, [[NP, ng], [16, 8], [1, 16]])

                    m1 = pp.tile([128, fd], f32, tag="m1" + sfx)
                    eng.tensor_mul(pw(m1), ei(EX), ej(EY))
                    m2 = pp.tile([128, fd], f32, tag="m2" + sfx)
                    eng.tensor_mul(pw(m2), ei(EY), ej(EX))
                    c = pp.tile([128, fd], f32, tag="c" + sfx)
                    eng.tensor_sub(c[:], m1[:], m2[:])
                    c2 = pp.tile([128, fd], f32, tag="c2" + sfx)
                    nc.scalar.activation(c2[:], c[:], AF.Square)
                    al = pp.tile([128, fd], f32, tag="al" + sfx)
                    eng.tensor_mul(pw(al), ej(WT), pw(c))
                    be = pp.tile([128, fd], f32, tag="be" + sfx)
                    eng.tensor_mul(pw(be), ei(WT), pw(c))
                    D = pp.tile([128, fd], f32, tag="D" + sfx)
                    eng.tensor_sub(D[:], al[:], be[:])
                    Y = pp.tile([128, fd], f32, tag="Y" + sfx)
                    eng.tensor_sub(Y[:], c2[:], D[:])
                    totp = pp.tile([128, fd], f32, tag="tp" + sfx)
                    eng.tensor_add(
                        pw(totp),
                        win(dw, g0 * W32, [[W32, ng], [0, 8], [1, 16]]),
                        win(dw, g0 * W32 + 1, [[W32, ng], [1, 8], [1, 16]]))
                    # min/pack chain: DVE only
                    v1 = pp.tile([128, fd], f32, tag="v1" + sfx)
                    nc.vector.scalar_tensor_tensor(
                        v1[:], be[:], -1.0, al[:], op.mult, op.min)
                    v2 = pp.tile([128, fd], f32, tag="v2" + sfx)
                    nc.vector.tensor_tensor(v2[:], v1[:], Y[:], op.min)
                    v3 = pp.tile([128, fd], f32, tag="v1" + sfx)
                    nc.vector.scalar_tensor_tensor(
                        v3[:], c2[:], TINY, v2[:], op.subtract, op.min)
                    ns = pp.tile([128, fd], f32, tag="v2" + sfx)
                    nc.vector.scalar_tensor_tensor(
                        ns[:], v3[:], -BIG, totp[:], op.mult, op.max)
                    nsi = ns[:].bitcast(i32)
                    ps2 = pp.tile([128, fd], i32, tag="ps" + sfx)
                    nc.vector.scalar_tensor_tensor(
                        win(ps2, 0, [[NP, ng], [1, NP]]),
                        win(nsi, 0, [[NP, ng], [1, NP]]),
                        cb[:, NP:NP + 1],
                        win(cb, 0, [[0, ng], [1, NP]]),
                        op.bitwise_and, op.bitwise_or)
                    ps2s.append((ps2, g0, ng))
                for ps2, g0, ng in ps2s:
                    nc.vector.tensor_reduce(
                        outsb[:, h * G + g0:h * G + g0 + ng],
                        win(ps2[:].bitcast(f32), 0, [[NP, ng], [1, NP]]),
                        axis=AX.X, op=op.min)

            def emit_store(vb):
                sl = slice(vb * 128, (vb + 1) * 128)
                nc.sync.dma_start(out_d[sl, :], outsbs[vb][:])

            # software-pipelined emission: heads run one pass ahead of bodies
            emit_load(0)
            st = {}
            st[(0, 0)] = emit_head(0, 0)
            st[(0, 1)] = emit_head(0, 1)
            emit_body(0, 0, st.pop((0, 0)))
            emit_load(1)
            st[(1, 0)] = emit_head(1, 0)
            emit_body(0, 1, st.pop((0, 1)))
            st[(1, 1)] = emit_head(1, 1)
            emit_store(0)
            emit_body(1, 0, st.pop((1, 0)))
            emit_body(1, 1, st.pop((1, 1)))
            emit_store(1)

    _cache["nc"] = nc
    return nc


def _in_maps(template, projections):
    tpl = np.ascontiguousarray(np.broadcast_to(
        np.asarray(template, dtype=np.float32).reshape(NRA * 2), (128, NRA * 2)))
    cst = _consts_np()
    maps = []
    for k in range(NCORES):
        shard = np.ascontiguousarray(
            projections[k * VS:(k + 1) * VS], dtype=np.float32)
        maps.append({"proj": shard, "tpl": tpl, "cst": cst})
    return maps


def _decode(raw, template, projections):
    """raw: [V, 40] f32(bits) packed-argmin records -> (weights, indices)."""
    rmin = np.ascontiguousarray(
        np.asarray(raw, dtype=np.float32)).view(np.int32).reshape(V, R, A)
    q = rmin & 127
    nsc = (rmin & ~np.int32(127)).view(np.float32)
    flag = nsc < 1e3

    k_sel = (q >> 4) + 1
    i_sel = q & 15
    j_sel = (i_sel + k_sel) % 16

    proj64 = np.asarray(projections, dtype=np.float64)
    tpl64 = np.asarray(template, dtype=np.float64)
    px64 = proj64[:, :, 0]
    py64 = proj64[:, :, 1]

    dxa = tpl64[None, :, :, 0:1] - px64[:, None, None, :]
    dya = tpl64[None, :, :, 1:2] - py64[:, None, None, :]
    dall = np.sqrt(dxa * dxa + dya * dya)            # (V,R,A,16)
    cidx = np.argmin(dall, axis=-1)                  # reference order[...,0]

    d_i = np.take_along_axis(dall, i_sel[..., None], -1)[..., 0]
    d_j = np.take_along_axis(dall, j_sel[..., None], -1)[..., 0]
    swap = (d_j < d_i) | ((d_j == d_i) & (j_sel < i_sel))
    first = np.where(swap, j_sel, i_sel)
    second = np.where(swap, i_sel, j_sel)

    def gat(arr, idx):
        return np.take_along_axis(arr, idx.reshape(V, -1), -1).reshape(V, R, A)

    Cx, Cy = gat(px64, cidx), gat(py64, cidx)
    e1x = gat(px64, first) - Cx
    e1y = gat(py64, first) - Cy
    e2x = gat(px64, second) - Cx
    e2y = gat(py64, second) - Cy
    vtx = tpl64[None, :, :, 0] - Cx
    vty = tpl64[None, :, :, 1] - Cy
    dot00 = e1x * e1x + e1y * e1y
    dot11 = e2x * e2x + e2y * e2y
    dot01 = e1x * e2x + e1y * e2y
    dot02 = e1x * vtx + e1y * vty
    dot12 = e2x * vtx + e2y * vty
    denom = dot00 * dot11 - dot01 * dot01
    with np.errstate(divide="ignore", invalid="ignore"):
        dinv = np.where(denom == 0.0, 0.0,
                        1.0 / np.where(denom == 0.0, 1.0, denom))
    p2 = (dot02 * dot11 - dot01 * dot12) * dinv
    p1 = (dot00 * dot12 - dot01 * dot02) * dinv
    p0 = 1.0 - p2 - p1

    ok = (flag & np.isfinite(p0) & np.isfinite(p1) & np.isfinite(p2)
          & (denom != 0.0))
    weights = np.zeros((V, R, A, 3), np.float32)
    indices = np.zeros((V, R, A, 3), np.int32)
    weights[..., 0] = np.where(ok, p0, 0).astype(np.float32)
    weights[..., 1] = np.where(ok, p2, 0).astype(np.float32)
    weights[..., 2] = np.where(ok, p1, 0).astype(np.float32)
    indices[..., 0] = np.where(ok, cidx, 0)
    indices[..., 1] = np.where(ok, first, 0)
    indices[..., 2] = np.where(ok, second, 0)
    return weights, indices


def _run_device(template, projections, trace=False, **kwargs):
    from concourse.bass_utils import run_bass_kernel_spmd
    nc = _build()
    if not _cache.get("legalized"):
        _legalize_waits(nc)
        _cache["legalized"] = True
    maps = _in_maps(template, projections)
    res = run_bass_kernel_spmd(nc, maps, core_ids=list(range(NCORES)),
                               trace=trace, **kwargs)
    raw = np.concatenate([r["out"] for r in res.results], axis=0)  # [V, 40]
    return raw, res


def kernel(template, projections):
    template = np.asarray(template, dtype=np.float32)
    projections = np.asarray(projections, dtype=np.float32)
    raw, _ = _run_device(template, projections, trace=False)
    return _decode(raw, template, projections)
